# revision 3
# baseline (speedup 1.0000x reference)
"""Distributed CFGCN propagate_embedding kernel for 8 TRN2 NeuronCores.

Strategy (1D dst-partitioning + int16 source bands):
  - Nodes split into 8 slices of N/8; core d owns slice d and the segment
    sums for its destinations. A replicated table of scaled features
    (x * sqrt_degree) lives in DRAM, rebuilt per layer with an AllGather.
  - Per layer each core gathers the source rows of its ~E/8 edges with
    `dma_gather` (int16 indices, SWDGE "Ant" path, ~8 ns/row, 4 queues
    round-robin). int16 reach forces the table into 5 source *bands* of
    <=32767 rows; edges are grouped by band.
  - Within a band, destination slots are re-sorted by band in-degree so
    round r (the r-th band-edge of every slot) is a dense prefix of
    columns: gathered tiles accumulate into a [128 x cols] SBUF acc via
    plain DVE adds (round 0 is a copy, so no memset).
  - Each band's acc (in band slot order) is merged into the canonical
    per-layer aggregate A_l in DRAM via `dma_scatter_add` with a
    permutation index vector (distinct targets -> no RMW races).
  - Layer tail: A_l is read back (canonical [p, c] layout), accumulated
    into the running output sum, scaled by sqrt_degree^2 and flushed as
    the AllGather input for the next layer's table.
  - Host applies the final ego add + sqrt_degree scale + /4 mean.
"""

import numpy as np

N_CORES = 8
P = 128
NB = 5            # int16 source bands
CHC = 96          # gather-call width in columns (96*128 = 12288 idx < ring cap)
NBUF = 2          # gather double-buffers
NQ = 4            # SWDGE queues (each served by its own Q7 core pair)


def _wrap16(a):
    """[n] -> [16, n//16] int16: index i at (i%16, i//16)."""
    return np.ascontiguousarray(np.asarray(a, np.int16).reshape(-1, 16).T)


def _build_schedule(emb, sqrt_degree, src, dst):
    n, d_model = emb.shape
    npc = n // N_CORES
    w = -(-npc // P)
    if w * P == npc:
        w += 1
    slots = w * P
    trows = N_CORES * slots
    band_rows = -(-trows // NB)
    assert band_rows <= 32767

    deg = np.bincount(dst, minlength=n)

    orders = []
    table_row_of = np.empty(n, dtype=np.int64)
    for d in range(N_CORES):
        lo = d * npc
        order = np.argsort(-deg[lo:lo + npc], kind="stable")
        orders.append(order)
        rank_of = np.empty(npc, dtype=np.int64)
        rank_of[order] = np.arange(npc)
        table_row_of[lo:lo + npc] = d * slots + rank_of

    band_zrow = np.empty(NB, dtype=np.int64)
    for b in range(NB):
        lo_r, hi_r = b * band_rows, min((b + 1) * band_rows, trows)
        zs = None
        for k in range(N_CORES):
            zr = k * slots + npc
            if lo_r <= zr < hi_r:
                zs = zr - lo_r
                break
        assert zs is not None
        band_zrow[b] = zs

    percore = []
    cmat = np.zeros((N_CORES, NB, 64), dtype=np.int64)
    rmax_db = np.zeros((N_CORES, NB), dtype=np.int64)
    for d in range(N_CORES):
        lo = d * npc
        mask = (dst >= lo) & (dst < lo + npc)
        es = table_row_of[src[mask].astype(np.int64)]
        er = table_row_of[dst[mask].astype(np.int64)] - d * slots
        eb = es // band_rows
        el = es - eb * band_rows
        bands = []
        for b in range(NB):
            m = eb == b
            r_sl, r_loc = er[m], el[m]
            X = np.bincount(r_sl, minlength=slots)
            pi = np.argsort(-X, kind="stable")
            rank_of = np.empty(slots, dtype=np.int64)
            rank_of[pi] = np.arange(slots)
            q = rank_of[r_sl]
            o = np.argsort(q, kind="stable")
            q_s, l_s = q[o], r_loc[o]
            starts = np.searchsorted(q_s, np.arange(slots), side="left")
            occ = np.arange(len(q_s)) - starts[q_s]
            rmax = int(occ.max()) + 1 if len(occ) else 1
            rmax_db[d, b] = rmax
            Xs = X[pi]
            for r in range(rmax):
                cmat[d, b, r] = int((Xs > r).sum())
            bands.append(dict(pi=pi, q=q_s, occ=occ, lidx=l_s))
        percore.append(bands)

    rmax_b = rmax_db.max(axis=0)
    cols = []
    for b in range(NB):
        cb = cmat[:, b, :rmax_b[b]].max(axis=0)
        cols.append(np.maximum(-(-cb // P), 1).astype(np.int64))

    # global column stream in (band, round) order
    col0 = {}
    c0 = 0
    band_c0 = np.zeros(NB + 1, dtype=np.int64)
    for b in range(NB):
        band_c0[b] = c0
        for r in range(int(rmax_b[b])):
            col0[(b, r)] = c0
            c0 += int(cols[b][r])
    band_c0[NB] = c0
    tot_cols = c0

    # gather calls: per band, cut its column range into chunks of <= CHC
    # cols; each call carries its DVE round-segments.
    calls = []   # (band, c_lo_global, ncols, [(op, acc_c0, buf_c0, n)])
    for b in range(NB):
        blo, bhi = int(band_c0[b]), int(band_c0[b + 1])
        c = blo
        while c < bhi:
            ck = min(CHC, bhi - c)
            segs = []
            for r in range(int(rmax_b[b])):
                g0, g1 = col0[(b, r)], col0[(b, r)] + int(cols[b][r])
                is_, ie = max(g0, c), min(g1, c + ck)
                if is_ < ie:
                    segs.append((0 if r == 0 else 1, is_ - g0, is_ - c,
                                 ie - is_))
            calls.append((b, c, ck, segs))
            c += ck

    # per-core gather idx stream, int16 band-local
    gidx = np.full((N_CORES, tot_cols * P), -1, dtype=np.int32)
    for d in range(N_CORES):
        for b in range(NB):
            bd = percore[d][b]
            for r in range(int(rmax_b[b])):
                s, e = col0[(b, r)] * P, (col0[(b, r)] + int(cols[b][r])) * P
                gidx[d, s:e] = band_zrow[b]
            base = np.array([col0[(b, r)] for r in range(int(rmax_b[b]))],
                            dtype=np.int64)
            p_ = base[bd["occ"]] * P + bd["q"]
            gidx[d, p_] = bd["lidx"]
    assert gidx.min() >= 0 and gidx.max() < band_rows

    # per-core scatter idx (band slot-permutation, distinct)
    scols = [int(cols[b][0]) for b in range(NB)]
    s_off = np.zeros(NB + 1, dtype=np.int64)
    for b in range(NB):
        s_off[b + 1] = s_off[b] + scols[b] * P
    sidx = np.empty((N_CORES, int(s_off[NB])), dtype=np.int32)
    for d in range(N_CORES):
        for b in range(NB):
            pi = percore[d][b]["pi"]
            sidx[d, s_off[b]:s_off[b + 1]] = pi[:scols[b] * P]
    assert sidx.min() >= 0 and sidx.max() < slots

    sd = np.asarray(sqrt_degree, np.float32).reshape(-1)
    scaled0, sd2_tiles = [], []
    for d in range(N_CORES):
        lo = d * npc
        order = orders[d]
        s0 = np.zeros((slots, d_model), dtype=np.float32)
        s0[:npc] = (np.asarray(emb[lo:lo + npc], np.float32)
                    * sd[lo:lo + npc, None])[order]
        scaled0.append(s0)
        s2 = np.zeros(slots, dtype=np.float32)
        s2[:npc] = (sd[lo:lo + npc] ** 2)[order]
        s2_pc = s2.reshape(w, P).T
        sd2_tiles.append(np.repeat(s2_pc, d_model, axis=1).astype(np.float32))

    return dict(
        n=n, d=d_model, npc=npc, w=w, slots=slots, trows=trows,
        band_rows=band_rows, orders=orders, rmax_b=rmax_b, cols=cols,
        col0=col0, band_c0=band_c0, tot_cols=tot_cols, calls=calls,
        gidx=gidx, sidx=sidx, scols=scols, s_off=s_off,
        scaled0=scaled0, sd2_tiles=sd2_tiles,
    )


def _build_program(S, n_layers):
    from concourse import bacc, mybir, tile

    f32 = mybir.dt.float32
    i16 = mybir.dt.int16
    D = S["d"]
    W = S["w"]
    SLOTS = S["slots"]
    TROWS = S["trows"]
    BR = S["band_rows"]
    TGI16 = S["tot_cols"] * P // 16
    TSI16 = int(S["s_off"][NB]) // 16

    nc = bacc.Bacc("TRN2", target_bir_lowering=False, debug=False,
                   num_devices=N_CORES, num_swdge_queues=NQ)
    scaled0_in = nc.dram_tensor("scaled0", [SLOTS, D], f32, kind="ExternalInput")
    sd2_in = nc.dram_tensor("sd2", [P, W * D], f32, kind="ExternalInput")
    gidx_in = nc.dram_tensor("gidx", [16, TGI16], i16, kind="ExternalInput")
    sidx_in = nc.dram_tensor("sidx", [16, TSI16], i16, kind="ExternalInput")
    out_d = nc.dram_tensor("out", [P, W * D], f32, kind="ExternalOutput")

    with tile.TileContext(nc) as tc:
        with tc.tile_pool(name="dram", bufs=1, space="DRAM") as dp, \
             tc.tile_pool(name="sb", bufs=1) as sb:
            tables = [dp.tile([TROWS, D], f32, addr_space="Shared",
                              name=f"tbl{l}") for l in range(n_layers)]
            A = [dp.tile([SLOTS, D], f32, name=f"agg{l}")
                 for l in range(n_layers)]
            S_in = [dp.tile([SLOTS, D], f32, name=f"sin{l}")
                    for l in range(n_layers)]
            gidx_rep = dp.tile([P, TGI16], i16, name="gidx_rep")

            sidx_sb = sb.tile([P, TSI16], i16, name="sidx_sb")
            sd2_sb = sb.tile([P, W * D], f32, name="sd2_sb")
            acc = sb.tile([P, W * D], f32, name="acc")
            mean = sb.tile([P, W * D], f32, name="mean")
            gbufs = [sb.tile([P, CHC, D], f32, name=f"gb{i}")
                     for i in range(NBUF)]
            ibufs = [sb.tile([P, CHC * 8], i16, name=f"ib{i}")
                     for i in range(NBUF)]

            # setup: replicate idx streams, load sd2, zero the aggregates
            for k in range(8):
                nc.sync.dma_start(out=gidx_rep[16 * k:16 * (k + 1), :],
                                  in_=gidx_in[:])
                nc.sync.dma_start(out=sidx_sb[16 * k:16 * (k + 1), :],
                                  in_=sidx_in[:])
            nc.sync.dma_start(out=sd2_sb[:], in_=sd2_in[:])
            nc.vector.memset(mean[:], 0.0)
            for l in range(n_layers):
                nc.sync.dma_start(
                    out=A[l][:].rearrange("(c p) f -> p c f", p=P),
                    in_=mean[:].rearrange("p (c f) -> p c f", f=D))

            # initial table: AllGather of host-computed scaled features
            nc.gpsimd.dma_start(out=S_in[0][:], in_=scaled0_in[:])
            nc.gpsimd.collective_compute(
                "AllGather", mybir.AluOpType.bypass,
                replica_groups=[list(range(N_CORES))],
                ins=[S_in[0][:]], outs=[tables[0][:, :]])

            qn = 0
            for l in range(n_layers):
                T = tables[l]
                cur_band = -1
                for (b, c_lo, ck, segs) in S["calls"]:
                    if b != cur_band and cur_band >= 0:
                        _scatter_band(nc, S, cur_band, acc, sidx_sb, A[l],
                                      D, qn % NQ)
                        qn += 1
                    cur_band = b
                    k = qn
                    ib = ibufs[k % NBUF]
                    gb = gbufs[k % NBUF]
                    nc.sync.dma_start(
                        out=ib[:, :ck * 8],
                        in_=gidx_rep[:, c_lo * 8:(c_lo + ck) * 8])
                    lo_r = b * BR
                    hi_r = min((b + 1) * BR, TROWS)
                    nc.gpsimd.dma_gather(
                        gb[:, :ck, :], T[lo_r:hi_r, :], ib[:, :ck * 8],
                        ck * P, ck * P, D, single_packet=False,
                        queue_num=k % NQ)
                    for (op, a0, b0, nseg) in segs:
                        dst = acc[:, a0 * D:(a0 + nseg) * D]
                        srcv = gb[:, b0:b0 + nseg, :].rearrange(
                            "p c f -> p (c f)")
                        if op == 0:
                            nc.vector.tensor_copy(out=dst, in_=srcv)
                        else:
                            nc.vector.tensor_add(out=dst, in0=dst, in1=srcv)
                    qn += 1
                _scatter_band(nc, S, cur_band, acc, sidx_sb, A[l], D, qn % NQ)
                qn += 1

                # layer tail: readback, mean accumulate, rescale, AllGather
                nc.sync.dma_start(
                    out=acc[:].rearrange("p (c f) -> p c f", f=D),
                    in_=A[l][:].rearrange("(c p) f -> p c f", p=P))
                nc.vector.tensor_add(out=mean[:], in0=mean[:], in1=acc[:])
                if l + 1 < n_layers:
                    nc.vector.tensor_tensor(
                        out=acc[:], in0=acc[:], in1=sd2_sb[:],
                        op=mybir.AluOpType.mult)
                    nc.sync.dma_start(
                        out=S_in[l + 1][:].rearrange("(c p) f -> p c f", p=P),
                        in_=acc[:].rearrange("p (c f) -> p c f", f=D))
                    nc.gpsimd.collective_compute(
                        "AllGather", mybir.AluOpType.bypass,
                        replica_groups=[list(range(N_CORES))],
                        ins=[S_in[l + 1][:]], outs=[tables[l + 1][:, :]])
            nc.sync.dma_start(out=out_d[:], in_=mean[:])
    nc.compile()
    return nc


def _scatter_band(nc, S, b, acc, sidx_sb, A_l, D, qn):
    """Merge band-b acc (band slot order) into A_l via scatter-adds.

    Scatter-add emits num_idxs/8 m2s descriptors (2 source reads per idx);
    the SWDGE ring holds 1024, so each call is capped at 60 cols (7680 idx).
    """
    sc = S["scols"][b]
    soff16 = int(S["s_off"][b]) // 16
    h0 = 0
    while h0 < sc:
        h = min(60, sc - h0)
        nc.gpsimd.dma_scatter_add(
            A_l[:, :],
            acc[:, h0 * D:(h0 + h) * D].rearrange("p (c f) -> p c f", f=D),
            sidx_sb[:, soff16 + h0 * 8:soff16 + (h0 + h) * 8],
            h * P, h * P, D, single_packet=False, queue_num=qn)
        h0 += h


def kernel(**inputs):
    emb = np.ascontiguousarray(np.asarray(inputs["emb"], dtype=np.float32))
    sqrt_degree = np.ascontiguousarray(
        np.asarray(inputs["sqrt_degree"], dtype=np.float32))
    src = np.asarray(inputs["src"], dtype=np.int32)
    dst = np.asarray(inputs["dst"], dtype=np.int32)
    n_layers = 3

    S = _build_schedule(emb, sqrt_degree, src, dst)
    nc = _build_program(S, n_layers)

    from concourse.bass_utils import run_bass_kernel_spmd
    in_maps = [
        {"scaled0": S["scaled0"][d],
         "sd2": S["sd2_tiles"][d],
         "gidx": _wrap16(S["gidx"][d]),
         "sidx": _wrap16(S["sidx"][d])}
        for d in range(N_CORES)
    ]
    res = run_bass_kernel_spmd(nc, in_maps, list(range(N_CORES)))

    n_nodes, d_model = emb.shape
    npc = S["npc"]
    W = S["w"]
    out = np.empty((n_nodes, d_model), dtype=np.float32)
    sd = sqrt_degree.reshape(-1)
    for d in range(N_CORES):
        lo = d * npc
        dev = res.results[d]["out"]                     # [P, W*D] slot layout
        agg = dev.reshape(P, W, d_model).transpose(1, 0, 2).reshape(-1, d_model)
        order = S["orders"][d]
        loc = np.empty((npc, d_model), dtype=np.float32)
        loc[order] = agg[:npc]
        out[lo:lo + npc] = (emb[lo:lo + npc]
                            + sd[lo:lo + npc, None] * loc) / (n_layers + 1)
    return out


# revision 7
# speedup vs baseline: 23.9084x; 23.9084x over previous
"""Distributed CFGCN propagate_embedding kernel for 8 TRN2 NeuronCores.

Strategy (1D dst-partitioning + int16 source bands):
  - Nodes split into 8 slices of N/8; core d owns slice d and the segment
    sums for its destinations. A replicated table of scaled features
    (x * sqrt_degree) lives in DRAM, rebuilt per layer with an AllGather.
  - Per layer each core gathers the source rows of its ~E/8 edges with
    `dma_gather` (int16 indices, SWDGE "Ant" path, 4 queues round-robin,
    4 rotating buffers). int16 reach forces the table into 5 source
    *bands* of <=32767 rows; edges are grouped by band.
  - Within a band, destination slots are re-sorted by band in-degree so
    round r (the r-th band-edge of every slot) is a dense prefix of
    columns: gathered tiles accumulate into a [128 x cols] SBUF acc via
    DVE adds (round 0 is a copy, so no memset).
  - Each band's acc (band slot order) is flushed to a DRAM staging strip
    with a plain sequential DMA; after all bands, 5 *merge gathers*
    (again int16 dma_gather, rank-permutation indices) re-read the strips
    in canonical slot order and DVE-sum them. This replaces a
    dma_scatter_add merge: gather descriptors are ~4x cheaper than
    scatter RMW descriptors.
  - Layer tail: merged acc joins the running output sum, is scaled by
    sqrt_degree^2 and flushed as the AllGather input for the next table.
  - Host applies the final ego add + sqrt_degree scale + /4 mean.
"""

import os
import numpy as np

N_CORES = 8
P = 128
NB = 5            # int16 source bands
CHC = 64          # gather-call width in columns (64*128 = 8192 idx)
NBUF = 4          # rotating gather buffers (keeps all 4 queue rings busy)
NQ = 4            # SWDGE queues (each served by its own Q7 core pair)


def _wrap16(a):
    """[n] -> [16, n//16] int16: index i at (i%16, i//16)."""
    return np.ascontiguousarray(np.asarray(a, np.int16).reshape(-1, 16).T)


def _build_schedule(emb, sqrt_degree, src, dst):
    n, d_model = emb.shape
    npc = n // N_CORES
    w = -(-npc // P)
    if w * P == npc:
        w += 1
    slots = w * P
    trows = N_CORES * slots
    band_rows = -(-trows // NB)
    assert band_rows <= 32767

    deg = np.bincount(dst, minlength=n)

    orders = []
    table_row_of = np.empty(n, dtype=np.int64)
    for d in range(N_CORES):
        lo = d * npc
        order = np.argsort(-deg[lo:lo + npc], kind="stable")
        orders.append(order)
        rank_of = np.empty(npc, dtype=np.int64)
        rank_of[order] = np.arange(npc)
        table_row_of[lo:lo + npc] = d * slots + rank_of

    band_zrow = np.empty(NB, dtype=np.int64)
    for b in range(NB):
        lo_r, hi_r = b * band_rows, min((b + 1) * band_rows, trows)
        zs = None
        for k in range(N_CORES):
            zr = k * slots + npc
            if lo_r <= zr < hi_r:
                zs = zr - lo_r
                break
        assert zs is not None
        band_zrow[b] = zs

    percore = []
    cmat = np.zeros((N_CORES, NB, 64), dtype=np.int64)
    rmax_db = np.zeros((N_CORES, NB), dtype=np.int64)
    for d in range(N_CORES):
        lo = d * npc
        mask = (dst >= lo) & (dst < lo + npc)
        es = table_row_of[src[mask].astype(np.int64)]
        er = table_row_of[dst[mask].astype(np.int64)] - d * slots
        eb = es // band_rows
        el = es - eb * band_rows
        bands = []
        for b in range(NB):
            m = eb == b
            r_sl, r_loc = er[m], el[m]
            X = np.bincount(r_sl, minlength=slots)
            pi = np.argsort(-X, kind="stable")
            rank_of = np.empty(slots, dtype=np.int64)
            rank_of[pi] = np.arange(slots)
            q = rank_of[r_sl]
            o = np.argsort(q, kind="stable")
            q_s, l_s = q[o], r_loc[o]
            starts = np.searchsorted(q_s, np.arange(slots), side="left")
            occ = np.arange(len(q_s)) - starts[q_s]
            rmax = int(occ.max()) + 1 if len(occ) else 1
            rmax_db[d, b] = rmax
            Xs = X[pi]
            for r in range(rmax):
                cmat[d, b, r] = int((Xs > r).sum())
            bands.append(dict(rank_of=rank_of, nact=int(cmat[d, b, 0]),
                              q=q_s, occ=occ, lidx=l_s))
        percore.append(bands)

    rmax_b = rmax_db.max(axis=0)
    cols = []
    for b in range(NB):
        cb = cmat[:, b, :rmax_b[b]].max(axis=0).copy()
        cb[0] += 1   # guarantees a zero row inside the staging strip
        cols.append(np.maximum(-(-cb // P), 1).astype(np.int64))
    scols = [int(cols[b][0]) for b in range(NB)]     # staging cols per band

    # global column stream in (band, round) order
    col0 = {}
    c0 = 0
    band_c0 = np.zeros(NB + 1, dtype=np.int64)
    for b in range(NB):
        band_c0[b] = c0
        for r in range(int(rmax_b[b])):
            col0[(b, r)] = c0
            c0 += int(cols[b][r])
    band_c0[NB] = c0
    tot_cols = c0

    # edge-gather calls: per band, cut its columns into chunks of <= CHC
    calls = []   # (band, c_lo_global, ncols, [(op, acc_c0, buf_c0, n)])
    for b in range(NB):
        blo, bhi = int(band_c0[b]), int(band_c0[b + 1])
        c = blo
        while c < bhi:
            ck = min(CHC, bhi - c)
            segs = []
            for r in range(int(rmax_b[b])):
                g0, g1 = col0[(b, r)], col0[(b, r)] + int(cols[b][r])
                is_, ie = max(g0, c), min(g1, c + ck)
                if is_ < ie:
                    segs.append((0 if r == 0 else 1, is_ - g0, is_ - c,
                                 ie - is_))
            calls.append((b, c, ck, segs))
            c += ck

    # per-core edge-gather idx stream, int16 band-local
    gidx = np.full((N_CORES, tot_cols * P), -1, dtype=np.int32)
    for d in range(N_CORES):
        for b in range(NB):
            bd = percore[d][b]
            for r in range(int(rmax_b[b])):
                s, e = col0[(b, r)] * P, (col0[(b, r)] + int(cols[b][r])) * P
                gidx[d, s:e] = band_zrow[b]
            base = np.array([col0[(b, r)] for r in range(int(rmax_b[b]))],
                            dtype=np.int64)
            p_ = base[bd["occ"]] * P + bd["q"]
            gidx[d, p_] = bd["lidx"]
    assert gidx.min() >= 0 and gidx.max() < band_rows

    # merge-gather calls: per band, read its staging strip (band slot
    # order) back in canonical slot order; chunks of <= CHC cols.
    mcalls = []  # (band, m_lo_cols, ncols, op)  op: 0 copy (first band) 1 add
    for b in range(NB):
        c = 0
        while c < w:
            ck = min(CHC, w - c)
            mcalls.append((b, c, ck, 0 if b == 0 else 1))
            c += ck

    # per-core merge idx: canonical slot s -> band-b staging rank (or a
    # known-zero staged row for slots with no band-b edges beyond the strip).
    midx = np.empty((N_CORES, NB * slots), dtype=np.int32)
    for d in range(N_CORES):
        for b in range(NB):
            bd = percore[d][b]
            lim = scols[b] * P
            rk = bd["rank_of"].copy()                 # canonical slot -> rank
            assert bd["nact"] < lim
            rk[rk >= lim] = bd["nact"]                # a zero staged row
            midx[d, b * slots:(b + 1) * slots] = rk
    assert midx.min() >= 0
    for b in range(NB):
        assert midx[:, b * slots:(b + 1) * slots].max() < scols[b] * P

    sd = np.asarray(sqrt_degree, np.float32).reshape(-1)
    scaled0, sd2_tiles = [], []
    for d in range(N_CORES):
        lo = d * npc
        order = orders[d]
        s0 = np.zeros((slots, d_model), dtype=np.float32)
        s0[:npc] = (np.asarray(emb[lo:lo + npc], np.float32)
                    * sd[lo:lo + npc, None])[order]
        scaled0.append(s0)
        s2 = np.zeros(slots, dtype=np.float32)
        s2[:npc] = (sd[lo:lo + npc] ** 2)[order]
        s2_pc = s2.reshape(w, P).T
        sd2_tiles.append(np.repeat(s2_pc, d_model, axis=1).astype(np.float32))

    return dict(
        n=n, d=d_model, npc=npc, w=w, slots=slots, trows=trows,
        band_rows=band_rows, orders=orders, rmax_b=rmax_b, cols=cols,
        col0=col0, band_c0=band_c0, tot_cols=tot_cols, calls=calls,
        mcalls=mcalls, gidx=gidx, midx=midx, scols=scols,
        scaled0=scaled0, sd2_tiles=sd2_tiles,
    )


def _build_program(S, n_layers):
    from concourse import bacc, mybir, tile

    f32 = mybir.dt.float32
    i16 = mybir.dt.int16
    D = S["d"]
    W = S["w"]
    SLOTS = S["slots"]
    TROWS = S["trows"]
    BR = S["band_rows"]
    scols = S["scols"]
    TGI16 = S["tot_cols"] * P // 16
    TMI16 = NB * SLOTS // 16

    nc = bacc.Bacc("TRN2", target_bir_lowering=False, debug=False,
                   num_devices=N_CORES, num_swdge_queues=NQ)
    scaled0_in = nc.dram_tensor("scaled0", [SLOTS, D], f32, kind="ExternalInput")
    sd2_in = nc.dram_tensor("sd2", [P, W * D], f32, kind="ExternalInput")
    gidx_in = nc.dram_tensor("gidx", [16, TGI16], i16, kind="ExternalInput")
    midx_in = nc.dram_tensor("midx", [16, TMI16], i16, kind="ExternalInput")
    out_d = nc.dram_tensor("out", [P, W * D], f32, kind="ExternalOutput")

    with tile.TileContext(nc) as tc:
        with tc.tile_pool(name="dram", bufs=1, space="DRAM") as dp, \
             tc.tile_pool(name="sb", bufs=1) as sb:
            S_in = [dp.tile([SLOTS, D], f32, name=f"sin{l}")
                    for l in range(n_layers)]
            staged = [dp.tile([scols[b] * P, D], f32, name=f"stg{b}")
                      for b in range(NB)]
            gidx_rep = dp.tile([P, TGI16], i16, name="gidx_rep")
            midx_rep = dp.tile([P, TMI16], i16, name="midx_rep")

            sd2_sb = sb.tile([P, W * D], f32, name="sd2_sb")
            acc = sb.tile([P, W * D], f32, name="acc")
            mean = sb.tile([P, W * D], f32, name="mean")
            gbufs = [sb.tile([P, CHC, D], f32, name=f"gb{i}")
                     for i in range(NBUF)]
            ibufs = [sb.tile([P, CHC * 8], i16, name=f"ib{i}")
                     for i in range(NBUF)]

            for k in range(8):
                nc.sync.dma_start(out=gidx_rep[16 * k:16 * (k + 1), :],
                                  in_=gidx_in[:])
                nc.sync.dma_start(out=midx_rep[16 * k:16 * (k + 1), :],
                                  in_=midx_in[:])
            nc.sync.dma_start(out=sd2_sb[:], in_=sd2_in[:])

            nrep = int(os.environ.get("KREPEAT", 1))
            for rep in range(nrep):
              tables = [dp.tile([TROWS, D], f32, addr_space="Shared",
                                name=f"tbl{rep}_{l}") for l in range(n_layers)]
              nc.vector.memset(mean[:], 0.0)

              # initial table: AllGather of host-computed scaled features
              nc.gpsimd.dma_start(out=S_in[0][:], in_=scaled0_in[:])
              nc.gpsimd.collective_compute(
                  "AllGather", mybir.AluOpType.bypass,
                  replica_groups=[list(range(N_CORES))],
                  ins=[S_in[0][:]], outs=[tables[0][:, :]])

              qn = 0
              for l in range(n_layers):
                T = tables[l]
                cur_band = -1
                edge_calls = ([] if os.environ.get("KNOGATH") else S["calls"])
                for (b, c_lo, ck, segs) in edge_calls:
                    if b != cur_band and cur_band >= 0:
                        _flush_band(nc, S, cur_band, acc, staged, D)
                    cur_band = b
                    k = qn
                    ib, gb = ibufs[k % NBUF], gbufs[k % NBUF]
                    nc.sync.dma_start(
                        out=ib[:, :ck * 8],
                        in_=gidx_rep[:, c_lo * 8:(c_lo + ck) * 8])
                    lo_r = b * BR
                    hi_r = min((b + 1) * BR, TROWS)
                    nc.gpsimd.dma_gather(
                        gb[:, :ck, :], T[lo_r:hi_r, :], ib[:, :ck * 8],
                        ck * P, ck * P, D, single_packet=False,
                        queue_num=k % 3)
                    if os.environ.get("KNODVE"):
                        segs = []
                    for (op, a0, b0, nseg) in segs:
                        dst = acc[:, a0 * D:(a0 + nseg) * D]
                        srcv = gb[:, b0:b0 + nseg, :].rearrange(
                            "p c f -> p (c f)")
                        if op == 0:
                            nc.vector.tensor_copy(out=dst, in_=srcv)
                        else:
                            nc.vector.tensor_add(out=dst, in0=dst, in1=srcv)
                    qn += 1
                if cur_band >= 0:
                    _flush_band(nc, S, cur_band, acc, staged, D)

                # merge: canonical-order gathers over the 5 staging strips
                mcalls = ([] if os.environ.get("KNOMERGE") else S["mcalls"])
                for (b, m_lo, ck, op) in mcalls:
                    k = qn
                    ib, gb = ibufs[k % NBUF], gbufs[k % NBUF]
                    i_lo = (b * SLOTS + m_lo * P) // 16
                    nc.sync.dma_start(
                        out=ib[:, :ck * 8],
                        in_=midx_rep[:, i_lo:i_lo + ck * 8])
                    nc.gpsimd.dma_gather(
                        gb[:, :ck, :], staged[b][:, :], ib[:, :ck * 8],
                        ck * P, ck * P, D, single_packet=False,
                        queue_num=k % 3)
                    dst = acc[:, m_lo * D:(m_lo + ck) * D]
                    srcv = gb[:, :ck, :].rearrange("p c f -> p (c f)")
                    if op == 0:
                        nc.vector.tensor_copy(out=dst, in_=srcv)
                    else:
                        nc.vector.tensor_add(out=dst, in0=dst, in1=srcv)
                    qn += 1

                # layer tail: mean accumulate, rescale, AllGather
                if not os.environ.get("KNOMERGE"):
                    nc.vector.tensor_add(out=mean[:], in0=mean[:], in1=acc[:])
                if l + 1 < n_layers:
                    nc.vector.tensor_tensor(
                        out=acc[:], in0=acc[:], in1=sd2_sb[:],
                        op=mybir.AluOpType.mult)
                    nc.sync.dma_start(
                        out=S_in[l + 1][:].rearrange("(c p) f -> p c f", p=P),
                        in_=acc[:].rearrange("p (c f) -> p c f", f=D))
                    nc.gpsimd.collective_compute(
                        "AllGather", mybir.AluOpType.bypass,
                        replica_groups=[list(range(N_CORES))],
                        ins=[S_in[l + 1][:]], outs=[tables[l + 1][:, :]])
            nc.sync.dma_start(out=out_d[:], in_=mean[:])
    nc.compile()
    return nc


def _flush_band(nc, S, b, acc, staged, D):
    """Plain sequential DMA of band-b acc columns to its staging strip."""
    sc = S["scols"][b]
    nc.sync.dma_start(
        out=staged[b][:].rearrange("(c p) f -> p c f", p=P),
        in_=acc[:, :sc * D].rearrange("p (c f) -> p c f", f=D))


def kernel(**inputs):
    emb = np.ascontiguousarray(np.asarray(inputs["emb"], dtype=np.float32))
    sqrt_degree = np.ascontiguousarray(
        np.asarray(inputs["sqrt_degree"], dtype=np.float32))
    src = np.asarray(inputs["src"], dtype=np.int32)
    dst = np.asarray(inputs["dst"], dtype=np.int32)
    n_layers = int(os.environ.get("KLAYERS", 3))

    S = _build_schedule(emb, sqrt_degree, src, dst)
    nc = _build_program(S, n_layers)

    from concourse.bass_utils import run_bass_kernel_spmd
    in_maps = [
        {"scaled0": S["scaled0"][d],
         "sd2": S["sd2_tiles"][d],
         "gidx": _wrap16(S["gidx"][d]),
         "midx": _wrap16(S["midx"][d])}
        for d in range(N_CORES)
    ]
    res = run_bass_kernel_spmd(nc, in_maps, list(range(N_CORES)))

    n_nodes, d_model = emb.shape
    npc = S["npc"]
    W = S["w"]
    out = np.empty((n_nodes, d_model), dtype=np.float32)
    sd = sqrt_degree.reshape(-1)
    for d in range(N_CORES):
        lo = d * npc
        dev = res.results[d]["out"]                     # [P, W*D] slot layout
        agg = dev.reshape(P, W, d_model).transpose(1, 0, 2).reshape(-1, d_model)
        order = S["orders"][d]
        loc = np.empty((npc, d_model), dtype=np.float32)
        loc[order] = agg[:npc]
        out[lo:lo + npc] = (emb[lo:lo + npc]
                            + sd[lo:lo + npc, None] * loc) / (n_layers + 1)
    return out
